# revision 17
# baseline (speedup 1.0000x reference)
"""Position-attention kernel for Trainium2 (8 NeuronCores, SPMD).

Reference computation (per batch b):
    q = Wq @ x + bq        [32, 4096]
    k = Wk @ x + bk        [32, 4096]
    v = Wv @ x + bv        [256, 4096]
    attn = softmax_j(q_i . k_j)           [4096, 4096]
    out[c, i] = sum_j v[c, j] attn[i, j]
    y = gamma * out + x

Sharding: B=4 batches x 2 query-halves -> 8 cores. Each core computes the
full softmax rows for its 2048 queries against all 4096 keys of its batch.
Host rotates x columns per core so the core's query half is always columns
0:2048 (softmax and the PV contraction are invariant to key/value column
order, as long as K and V use the same order).

Device-side structure (per core):
  - projections in bf16 (x pre-cast on host; weights pre-packed on host).
  - scores computed transposed (sT[j, i]) in PSUM, 2 key-blocks at a time
    packed into PE row-groups 0/32 via tile_position (K=32 contractions run
    concurrently); kf is stored packed ([d + 32*r] rows), q replicated 4x.
  - one exp (ACT) per 2 key-blocks: PSUM [128, 2, 512] -> SBUF bf16; these
    e-tiles persist for the whole query supertile.
  - PV: out[i, c] = sum_j e[j, i] * vT[j, c] with e-blocks as the stationary
    operand; vT carries an extra all-ones column so column 256 of the
    output is the softmax denominator (per-partition = per-query).
  - epilogue: y_T[i, :] = out[i, :] * (gamma / sum_i) + (x_T[i, :] +
    gamma * bv)  -- the bv term works because sum_j attn = 1; it is folded
    into the precomputed xpb tile. Output written transposed; host
    transposes back (pure layout).
"""

import os
import numpy as np

P = 128
B = 4
C = 256
CQ = 32
H = W = 64
N = H * W            # 4096 keys per batch
NH = N // 2          # 2048 queries per core
NCB = C // P         # 2 channel blocks
ST = 512             # query supertile
NST = NH // ST       # 4
JB = N // P          # 32 key blocks
NG = JB // 2         # 16 score groups (2 key blocks each)

_PROG = None         # cached build
LAST_RESULT = None   # BassKernelResults of the last run (for test harness)


def _build_program():
    import concourse.mybir as mybir
    import concourse.tile as tile
    from concourse import bacc
    from concourse.bass import ds

    fp32 = mybir.dt.float32
    bf16 = mybir.dt.bfloat16

    nc = bacc.Bacc(None, target_bir_lowering=False, debug=False)

    xb_d = nc.declare_dram_parameter("xb", [C, N], bf16, isOutput=False)
    # xpb = xT + gamma*bv, already in SBUF layout [p, row_block*C]
    xpb_d = nc.declare_dram_parameter("xpb", [P, (NH // P) * C], fp32, isOutput=False)
    wq_d = nc.declare_dram_parameter("wq_rep", [C, P], bf16, isOutput=False)
    wk_d = nc.declare_dram_parameter("wk_pack", [C, 2, P], bf16, isOutput=False)
    wv_d = nc.declare_dram_parameter("wvT", [C, C], bf16, isOutput=False)
    bq_d = nc.declare_dram_parameter("bq_rep", [P, 1], fp32, isOutput=False)
    bk_d = nc.declare_dram_parameter("bk_pack", [P, 1], fp32, isOutput=False)
    gm_d = nc.declare_dram_parameter("gamma_bc", [P, 1], fp32, isOutput=False)
    yT_d = nc.declare_dram_parameter("yT", [NH, C], fp32, isOutput=True)

    with tile.TileContext(nc) as tc:
        with (
            tc.tile_pool(name="singles", bufs=1) as singles,
            tc.tile_pool(name="epool", bufs=20) as epool,
            tc.tile_pool(name="stpool", bufs=4) as stpool,
            tc.tile_pool(name="ivpool", bufs=4) as ivpool,
            tc.tile_pool(name="pp_mm", bufs=3, space="PSUM") as pp_mm,
            tc.tile_pool(name="pp_out", bufs=2, space="PSUM") as pp_out,
        ):
            # ---- persistent SBUF tensors ----
            xb_sb = singles.tile([P, NCB, N], bf16)
            xpb_sb = singles.tile([P, NH // P, C], fp32)  # xT + gamma*bv
            wq_sb = singles.tile([P, NCB, P], bf16)
            wk_sb = singles.tile([P, NCB, 2, P], bf16)
            wv_sb = singles.tile([P, NCB, C], bf16)
            bq_sb = singles.tile([P, 1], fp32)
            bk_sb = singles.tile([P, 1], fp32)
            gm_sb = singles.tile([P, 1], fp32)
            kf_sb = singles.tile([P, NG, P], bf16)   # packed: row 32r+d, grp g
            q_sb = singles.tile([P, NH], bf16)       # q replicated in 4 groups
            vT_sb = singles.tile([P, JB, C + 1], bf16)  # col C is all-ones

            # ---- input DMAs. Per-queue BW is ~34 GB/s, so spread the big
            # tensors over many queues with >=2KB descriptors. wk first (the
            # K projection needs it), then x column-chunks in consumption
            # order, interleaved with the remaining weights.
            nc.sync.dma_start(
                out=wk_sb[:], in_=wk_d.rearrange("(o p) r m -> p o r m", p=P)
            )
            xpb_flat = xpb_sb.rearrange("p o c -> p (o c)")
            for cc in range(4):
                csl = ds(cc * 1024, 1024)
                for cb in range(NCB):
                    for rh in range(2):
                        rsl = slice(rh * 64, (rh + 1) * 64)
                        nc.sync.dma_start(
                            out=xb_sb[rsl, cb, csl],
                            in_=xb_d[cb * P + rh * 64:cb * P + (rh + 1) * 64, csl],
                        )
                if cc == 0:
                    nc.sync.dma_start(
                        out=wq_sb[:], in_=wq_d.rearrange("(o p) m -> p o m", p=P)
                    )
                    nc.sync.dma_start(out=bq_sb[:], in_=bq_d[:])
                    nc.sync.dma_start(out=bk_sb[:], in_=bk_d[:])
                    nc.sync.dma_start(out=gm_sb[:], in_=gm_d[:])
                elif cc == 1:
                    nc.sync.dma_start(
                        out=wv_sb[:], in_=wv_d.rearrange("(o p) m -> p o m", p=P)
                    )
            for rh in range(2):
                for cc in range(8):
                    rsl = slice(rh * 64, (rh + 1) * 64)
                    csl = ds(cc * 512, 512)
                    nc.sync.dma_start(
                        out=xpb_flat[rsl, csl], in_=xpb_d[rsl, csl]
                    )

            nc.vector.memset(vT_sb[:, :, C:C + 1], 1.0)

            # ---- projection helpers ----
            # kf_sb[32r + d, g, :] = (Wk @ x + bk)[d, (2g + r)*128 : ...]
            # wk variant r has WkT at column offset 32r (zeros elsewhere), so
            # the four accumulating matmuls write disjoint row blocks.
            def k_proj(g):
                kp = pp_mm.tile([P, P], fp32, tag="mm", name=f"kp_{g}")
                for i, (r, cb) in enumerate(
                    [(r, cb) for r in range(2) for cb in range(NCB)]
                ):
                    nc.tensor.matmul(
                        kp, wk_sb[:, cb, r], xb_sb[:, cb, ds((2 * g + r) * P, P)],
                        start=(i == 0), stop=(i == 3),
                    )
                nc.vector.tensor_scalar_add(kf_sb[:, g, :], kp, bk_sb)

            def v_proj(j):
                # uses the (otherwise idle during score phase) out pool
                vp = pp_out.tile([P, C], fp32, tag="out", name=f"vp_{j}")
                nc.tensor.matmul(
                    vp, xb_sb[:, 0, ds(j * P, P)], wv_sb[:, 0], start=True, stop=False
                )
                nc.tensor.matmul(
                    vp, xb_sb[:, 1, ds(j * P, P)], wv_sb[:, 1], start=False, stop=True
                )
                nc.vector.tensor_copy(vT_sb[:, j, 0:C], vp)

            # ---- Q projection chunk (replicated across the 4 row groups) ----
            def q_proj(t):
                qp = pp_mm.tile([P, ST], fp32, tag="mm", name=f"qp_{t}")
                nc.tensor.matmul(
                    qp, wq_sb[:, 0], xb_sb[:, 0, ds(t * ST, ST)],
                    start=True, stop=False,
                )
                nc.tensor.matmul(
                    qp, wq_sb[:, 1], xb_sb[:, 1, ds(t * ST, ST)],
                    start=False, stop=True,
                )
                nc.vector.tensor_scalar_add(q_sb[:, ds(t * ST, ST)], qp, bq_sb)

            # Minimal prefix: st0's first score group needs only kf g0 + q t0.
            k_proj(0)
            k_proj(1)
            q_proj(0)

            # ---- attention over query supertiles ----
            # st0's score/exp phase is ACT-paced; fill PE with the remaining
            # K/Q projections (just-in-time) and the whole V projection.
            for st_i in range(NST):
                es = []
                for g in range(NG):
                    if st_i == 0:
                        if g + 2 < NG:
                            k_proj(g + 2)
                        if g in (4, 8, 12):
                            q_proj(g // 4)
                    sps = pp_mm.tile([P, 2, ST], fp32, tag="mm")
                    for r in range(2):
                        nc.tensor.matmul(
                            sps[:, r],
                            kf_sb[32 * r:32 * (r + 1), g, :],
                            q_sb[32 * r:32 * (r + 1), ds(st_i * ST, ST)],
                            start=True, stop=True,
                            tile_position=(32 * r, 0),
                        )
                    e = epool.tile([P, 2, ST], bf16, name=f"e_{st_i}_{g}", tag="e")
                    nc.scalar.activation(e, sps, mybir.ActivationFunctionType.Exp)
                    es.append(e)
                    if st_i == 0:
                        v_proj(2 * g)
                        v_proj(2 * g + 1)
                for ib in range(4):
                    out_ps = pp_out.tile(
                        [P, C + 1], fp32, tag="out", name=f"out_{st_i}_{ib}"
                    )
                    for j in range(JB):
                        nc.tensor.matmul(
                            out_ps,
                            es[j // 2][:, j % 2, ds(ib * P, P)],
                            vT_sb[:, j, :],
                            start=(j == 0), stop=(j == JB - 1),
                        )
                    # epilogue: per-partition normalize + gamma + residual
                    row = st_i * 4 + ib
                    inv = ivpool.tile([P, 1], fp32)
                    nc.vector.reciprocal(inv, out_ps[:, C:C + 1])
                    nc.vector.tensor_scalar_mul(inv, inv, gm_sb)
                    stg = stpool.tile([P, C], fp32)
                    nc.vector.scalar_tensor_tensor(
                        stg, out_ps[:, 0:C], inv, xpb_sb[:, row, :],
                        op0=mybir.AluOpType.mult,
                        op1=mybir.AluOpType.add,
                    )
                    # split across two queues to halve the store latency
                    nc.sync.dma_start(
                        out=yT_d[ds(row * P, 64), :], in_=stg[0:64, :]
                    )
                    nc.sync.dma_start(
                        out=yT_d[ds(row * P + 64, 64), :], in_=stg[64:128, :]
                    )

    return nc


def _get_program():
    global _PROG
    if _PROG is None:
        _PROG = _build_program()
        if not _PROG.is_finalized():
            _PROG.finalize()
    return _PROG


def kernel(x, Wq, bq, Wk, bk, Wv, bv, gamma):
    global LAST_RESULT
    import ml_dtypes
    from concourse.bass_utils import run_bass_kernel_spmd

    bf16 = ml_dtypes.bfloat16
    x = np.ascontiguousarray(np.asarray(x, dtype=np.float32))
    Wq = np.asarray(Wq, dtype=np.float32)
    bq = np.asarray(bq, dtype=np.float32)
    Wk = np.asarray(Wk, dtype=np.float32)
    bk = np.asarray(bk, dtype=np.float32)
    Wv = np.asarray(Wv, dtype=np.float32)
    bv = np.asarray(bv, dtype=np.float32)
    gamma = np.asarray(gamma, dtype=np.float32)

    # wq replicated into all four 32-row groups of the PE array
    wq_rep = np.zeros((C, P), dtype=np.float32)
    for r in range(4):
        wq_rep[:, 32 * r:32 * (r + 1)] = Wq.T
    # wk variant r carries WkT at column offset 32r (r = 0, 1)
    wk_pack = np.zeros((C, 2, P), dtype=np.float32)
    for r in range(2):
        wk_pack[:, r, 32 * r:32 * (r + 1)] = Wk.T
    bq_rep = np.tile(bq, 4)[:, None].astype(np.float32)
    bk_pack = np.zeros((P, 1), dtype=np.float32)
    bk_pack[0:32, 0] = bk
    bk_pack[32:64, 0] = bk
    gval = float(gamma.reshape(-1)[0])
    gm_bc = np.full((P, 1), gval, dtype=np.float32)

    wq_rep = np.ascontiguousarray(wq_rep.astype(bf16))
    wk_pack = np.ascontiguousarray(wk_pack.astype(bf16))
    wvT = np.ascontiguousarray(Wv.T.astype(bf16))

    xf = x.reshape(B, C, N)
    in_maps = []
    for core in range(8):
        b, h = core // 2, core % 2
        xb = xf[b]
        if h == 0:
            x_roll = xb
        else:
            x_roll = np.concatenate([xb[:, NH:], xb[:, :NH]], axis=1)
        # xpb[p, o, c] = x_roll[c, o*128 + p] + gamma*bv[c]  (SBUF layout)
        xqT = x_roll[:, :NH].T + gval * bv[None, :]
        xpb = np.ascontiguousarray(
            xqT.reshape(NH // P, P, C).transpose(1, 0, 2).reshape(P, (NH // P) * C)
        ).astype(np.float32)
        in_maps.append({
            "xb": np.ascontiguousarray(x_roll.astype(bf16)),
            "xpb": xpb,
            "wq_rep": wq_rep,
            "wk_pack": wk_pack,
            "wvT": wvT,
            "bq_rep": bq_rep,
            "bk_pack": bk_pack,
            "gamma_bc": gm_bc,
        })

    nc = _get_program()
    res = run_bass_kernel_spmd(
        nc, in_maps, core_ids=list(range(8)),
        trace=bool(os.environ.get("BASS_TRACE")),
    )
    LAST_RESULT = res

    out = np.empty((B, C, N), dtype=np.float32)
    for core in range(8):
        b, h = core // 2, core % 2
        yT = res.results[core]["yT"]
        out[b][:, h * NH:(h + 1) * NH] = yT.T
    return out.reshape(B, C, H, W)


# revision 21
# speedup vs baseline: 1.0071x; 1.0071x over previous
"""Position-attention kernel for Trainium2 (8 NeuronCores, SPMD).

Reference computation (per batch b):
    q = Wq @ x + bq        [32, 4096]
    k = Wk @ x + bk        [32, 4096]
    v = Wv @ x + bv        [256, 4096]
    attn = softmax_j(q_i . k_j)           [4096, 4096]
    out[c, i] = sum_j v[c, j] attn[i, j]
    y = gamma * out + x

Sharding: B=4 batches x 2 query-halves -> 8 cores. Each core computes the
full softmax rows for its 2048 queries against all 4096 keys of its batch.
Host rotates x columns per core so the core's query half is always columns
0:2048 (softmax and the PV contraction are invariant to key/value column
order, as long as K and V use the same order).

Device-side structure (per core):
  - projections in bf16 (x pre-cast on host; weights pre-packed on host).
  - scores computed transposed (sT[j, i]) in PSUM, 2 key-blocks at a time
    packed into PE row-groups 0/32 via tile_position (K=32 contractions run
    concurrently); kf is stored packed ([d + 32*r] rows), q replicated 4x.
  - one exp (ACT) per 2 key-blocks: PSUM [128, 2, 512] -> SBUF bf16; these
    e-tiles persist for the whole query supertile.
  - PV: out[i, c] = sum_j e[j, i] * vT[j, c] with e-blocks as the stationary
    operand; vT carries an extra all-ones column so column 256 of the
    output is the softmax denominator (per-partition = per-query).
  - epilogue: y_T[i, :] = out[i, :] * (gamma / sum_i) + (x_T[i, :] +
    gamma * bv)  -- the bv term works because sum_j attn = 1; it is folded
    into the precomputed xpb tile. Output written transposed; host
    transposes back (pure layout).
"""

import os
import numpy as np

P = 128
B = 4
C = 256
CQ = 32
H = W = 64
N = H * W            # 4096 keys per batch
NH = N // 2          # 2048 queries per core
NCB = C // P         # 2 channel blocks
ST = 512             # query supertile
NST = NH // ST       # 4
JB = N // P          # 32 key blocks
NG = JB // 2         # 16 score groups (2 key blocks each)

_PROG = None         # cached build
LAST_RESULT = None   # BassKernelResults of the last run (for test harness)


def _build_program():
    import concourse.mybir as mybir
    import concourse.tile as tile
    from concourse import bacc
    from concourse.bass import ds

    fp32 = mybir.dt.float32
    bf16 = mybir.dt.bfloat16

    nc = bacc.Bacc(None, target_bir_lowering=False, debug=False)

    xb_d = nc.declare_dram_parameter("xb", [C, N], bf16, isOutput=False)
    # xpb = xT + gamma*bv, already in SBUF layout [p, row_block*C]
    xpb_d = nc.declare_dram_parameter("xpb", [P, (NH // P) * C], fp32, isOutput=False)
    wq_d = nc.declare_dram_parameter("wq_rep", [C, P], bf16, isOutput=False)
    wk_d = nc.declare_dram_parameter("wk_pack", [C, 2, P], bf16, isOutput=False)
    wv_d = nc.declare_dram_parameter("wvT", [C, C], bf16, isOutput=False)
    bq_d = nc.declare_dram_parameter("bq_rep", [P, 1], fp32, isOutput=False)
    bk_d = nc.declare_dram_parameter("bk_pack", [P, 1], fp32, isOutput=False)
    gm_d = nc.declare_dram_parameter("gamma_bc", [P, 1], fp32, isOutput=False)
    yT_d = nc.declare_dram_parameter("yT", [NH, C], fp32, isOutput=True)

    with tile.TileContext(nc) as tc:
        with (
            tc.tile_pool(name="singles", bufs=1) as singles,
            tc.tile_pool(name="epool", bufs=20) as epool,
            tc.tile_pool(name="stpool", bufs=4) as stpool,
            tc.tile_pool(name="ivpool", bufs=4) as ivpool,
            tc.tile_pool(name="pp_mm", bufs=3, space="PSUM") as pp_mm,
            tc.tile_pool(name="pp_out", bufs=2, space="PSUM") as pp_out,
        ):
            # ---- persistent SBUF tensors ----
            xb_sb = singles.tile([P, NCB, N], bf16)
            xpb_sb = singles.tile([P, NH // P, C], fp32)  # xT + gamma*bv
            wq_sb = singles.tile([P, NCB, P], bf16)
            wk_sb = singles.tile([P, NCB, 2, P], bf16)
            wv_sb = singles.tile([P, NCB, C], bf16)
            bq_sb = singles.tile([P, 1], fp32)
            bk_sb = singles.tile([P, 1], fp32)
            gm_sb = singles.tile([P, 1], fp32)
            kf_sb = singles.tile([P, NG, P], bf16)   # packed: row 32r+d, grp g
            q_sb = singles.tile([P, NH], bf16)       # q replicated in 4 groups
            vT_sb = singles.tile([P, JB, C + 1], bf16)  # col C is all-ones

            # ---- input DMAs. Per-queue BW is ~34 GB/s, so spread the big
            # tensors over many queues with >=2KB descriptors. wk first (the
            # K projection needs it), then x column-chunks in consumption
            # order, interleaved with the remaining weights.
            nc.sync.dma_start(
                out=wk_sb[:], in_=wk_d.rearrange("(o p) r m -> p o r m", p=P)
            )
            xpb_flat = xpb_sb.rearrange("p o c -> p (o c)")
            # first 1024 columns arrive as 512-col chunks (halves first-MM
            # latency); the rest as 1024-col chunks
            col_chunks = [(0, 512), (512, 512), (1024, 512), (1536, 512),
                          (2048, 1024), (3072, 1024)]
            for c0, cw in col_chunks:
                csl = ds(c0, cw)
                for cb in range(NCB):
                    for rh in range(2):
                        rsl = slice(rh * 64, (rh + 1) * 64)
                        nc.sync.dma_start(
                            out=xb_sb[rsl, cb, csl],
                            in_=xb_d[cb * P + rh * 64:cb * P + (rh + 1) * 64, csl],
                        )
                if c0 == 0:
                    nc.sync.dma_start(
                        out=wq_sb[:], in_=wq_d.rearrange("(o p) m -> p o m", p=P)
                    )
                    nc.sync.dma_start(out=bq_sb[:], in_=bq_d[:])
                    nc.sync.dma_start(out=bk_sb[:], in_=bk_d[:])
                    nc.sync.dma_start(out=gm_sb[:], in_=gm_d[:])
                elif c0 == 1024:
                    nc.sync.dma_start(
                        out=wv_sb[:], in_=wv_d.rearrange("(o p) m -> p o m", p=P)
                    )
            for rh in range(2):
                for cc in range(8):
                    rsl = slice(rh * 64, (rh + 1) * 64)
                    csl = ds(cc * 512, 512)
                    nc.sync.dma_start(
                        out=xpb_flat[rsl, csl], in_=xpb_d[rsl, csl]
                    )

            nc.vector.memset(vT_sb[:, :, C:C + 1], 1.0)

            # ---- boot-time warmup (runs while input DMAs are in flight) ----
            # 1) a dummy exp pre-loads the ACT function table (~2.7us) so the
            #    first real exp doesn't pay it; 2) dummy matmuls keep the PE
            #    HAM activity monitor busy so real matmuls start at 2.4 GHz.
            warm_sb = singles.tile([P, ST], bf16)
            warm_e = singles.tile([1, 1], fp32)
            nc.vector.memset(warm_sb[:], 0.0)
            nc.scalar.activation(
                warm_e, warm_sb[0:1, 0:1], mybir.ActivationFunctionType.Exp
            )
            for w in range(8):
                wp = pp_mm.tile([P, ST], fp32, tag="mm", name=f"warm_{w}")
                nc.tensor.matmul(
                    wp, warm_sb[:, 0:P], warm_sb, start=True, stop=True
                )

            # ---- projection helpers ----
            # kf_sb[32r + d, g, :] = (Wk @ x + bk)[d, (2g + r)*128 : ...]
            # wk variant r has WkT at column offset 32r (zeros elsewhere), so
            # the four accumulating matmuls write disjoint row blocks.
            def k_proj(g):
                kp = pp_mm.tile([P, P], fp32, tag="mm", name=f"kp_{g}")
                for i, (r, cb) in enumerate(
                    [(r, cb) for r in range(2) for cb in range(NCB)]
                ):
                    nc.tensor.matmul(
                        kp, wk_sb[:, cb, r], xb_sb[:, cb, ds((2 * g + r) * P, P)],
                        start=(i == 0), stop=(i == 3),
                    )
                nc.vector.tensor_scalar_add(kf_sb[:, g, :], kp, bk_sb)

            def v_proj(j):
                # uses the (otherwise idle during score phase) out pool
                vp = pp_out.tile([P, C], fp32, tag="out", name=f"vp_{j}")
                nc.tensor.matmul(
                    vp, xb_sb[:, 0, ds(j * P, P)], wv_sb[:, 0], start=True, stop=False
                )
                nc.tensor.matmul(
                    vp, xb_sb[:, 1, ds(j * P, P)], wv_sb[:, 1], start=False, stop=True
                )
                nc.vector.tensor_copy(vT_sb[:, j, 0:C], vp)

            # ---- Q projection chunk (replicated across the 4 row groups) ----
            def q_proj(t):
                qp = pp_mm.tile([P, ST], fp32, tag="mm", name=f"qp_{t}")
                nc.tensor.matmul(
                    qp, wq_sb[:, 0], xb_sb[:, 0, ds(t * ST, ST)],
                    start=True, stop=False,
                )
                nc.tensor.matmul(
                    qp, wq_sb[:, 1], xb_sb[:, 1, ds(t * ST, ST)],
                    start=False, stop=True,
                )
                nc.vector.tensor_scalar_add(q_sb[:, ds(t * ST, ST)], qp, bq_sb)

            # Minimal prefix: st0's first score group needs only kf g0 + q t0.
            k_proj(0)
            k_proj(1)
            q_proj(0)

            # ---- attention over query supertiles ----
            # st0's score/exp phase is ACT-paced; fill PE with the remaining
            # K/Q projections (just-in-time) and the whole V projection.
            for st_i in range(NST):
                es = []
                for g in range(NG):
                    if st_i == 0:
                        if g + 2 < NG:
                            k_proj(g + 2)
                        if g in (4, 8, 12):
                            q_proj(g // 4)
                    sps = pp_mm.tile([P, 2, ST], fp32, tag="mm")
                    for r in range(2):
                        nc.tensor.matmul(
                            sps[:, r],
                            kf_sb[32 * r:32 * (r + 1), g, :],
                            q_sb[32 * r:32 * (r + 1), ds(st_i * ST, ST)],
                            start=True, stop=True,
                            tile_position=(32 * r, 0),
                        )
                    e = epool.tile([P, 2, ST], bf16, name=f"e_{st_i}_{g}", tag="e")
                    nc.scalar.activation(e, sps, mybir.ActivationFunctionType.Exp)
                    es.append(e)
                    if st_i == 0:
                        v_proj(2 * g)
                        v_proj(2 * g + 1)
                for ib in range(4):
                    out_ps = pp_out.tile(
                        [P, C + 1], fp32, tag="out", name=f"out_{st_i}_{ib}"
                    )
                    for j in range(JB):
                        nc.tensor.matmul(
                            out_ps,
                            es[j // 2][:, j % 2, ds(ib * P, P)],
                            vT_sb[:, j, :],
                            start=(j == 0), stop=(j == JB - 1),
                        )
                    # epilogue: per-partition normalize + gamma + residual
                    row = st_i * 4 + ib
                    inv = ivpool.tile([P, 1], fp32)
                    nc.vector.reciprocal(inv, out_ps[:, C:C + 1])
                    nc.vector.tensor_scalar_mul(inv, inv, gm_sb)
                    stg = stpool.tile([P, C], fp32)
                    nc.vector.scalar_tensor_tensor(
                        stg, out_ps[:, 0:C], inv, xpb_sb[:, row, :],
                        op0=mybir.AluOpType.mult,
                        op1=mybir.AluOpType.add,
                    )
                    # split across four queues to cut the store latency
                    for rq in range(4):
                        nc.sync.dma_start(
                            out=yT_d[ds(row * P + rq * 32, 32), :],
                            in_=stg[rq * 32:(rq + 1) * 32, :],
                        )

    return nc


def _get_program():
    global _PROG
    if _PROG is None:
        _PROG = _build_program()
        if not _PROG.is_finalized():
            _PROG.finalize()
    return _PROG


def kernel(x, Wq, bq, Wk, bk, Wv, bv, gamma):
    global LAST_RESULT
    import ml_dtypes
    from concourse.bass_utils import run_bass_kernel_spmd

    bf16 = ml_dtypes.bfloat16
    x = np.ascontiguousarray(np.asarray(x, dtype=np.float32))
    Wq = np.asarray(Wq, dtype=np.float32)
    bq = np.asarray(bq, dtype=np.float32)
    Wk = np.asarray(Wk, dtype=np.float32)
    bk = np.asarray(bk, dtype=np.float32)
    Wv = np.asarray(Wv, dtype=np.float32)
    bv = np.asarray(bv, dtype=np.float32)
    gamma = np.asarray(gamma, dtype=np.float32)

    # wq replicated into all four 32-row groups of the PE array
    wq_rep = np.zeros((C, P), dtype=np.float32)
    for r in range(4):
        wq_rep[:, 32 * r:32 * (r + 1)] = Wq.T
    # wk variant r carries WkT at column offset 32r (r = 0, 1)
    wk_pack = np.zeros((C, 2, P), dtype=np.float32)
    for r in range(2):
        wk_pack[:, r, 32 * r:32 * (r + 1)] = Wk.T
    bq_rep = np.tile(bq, 4)[:, None].astype(np.float32)
    bk_pack = np.zeros((P, 1), dtype=np.float32)
    bk_pack[0:32, 0] = bk
    bk_pack[32:64, 0] = bk
    gval = float(gamma.reshape(-1)[0])
    gm_bc = np.full((P, 1), gval, dtype=np.float32)

    wq_rep = np.ascontiguousarray(wq_rep.astype(bf16))
    wk_pack = np.ascontiguousarray(wk_pack.astype(bf16))
    wvT = np.ascontiguousarray(Wv.T.astype(bf16))

    xf = x.reshape(B, C, N)
    in_maps = []
    for core in range(8):
        b, h = core // 2, core % 2
        xb = xf[b]
        if h == 0:
            x_roll = xb
        else:
            x_roll = np.concatenate([xb[:, NH:], xb[:, :NH]], axis=1)
        # xpb[p, o, c] = x_roll[c, o*128 + p] + gamma*bv[c]  (SBUF layout)
        xqT = x_roll[:, :NH].T + gval * bv[None, :]
        xpb = np.ascontiguousarray(
            xqT.reshape(NH // P, P, C).transpose(1, 0, 2).reshape(P, (NH // P) * C)
        ).astype(np.float32)
        in_maps.append({
            "xb": np.ascontiguousarray(x_roll.astype(bf16)),
            "xpb": xpb,
            "wq_rep": wq_rep,
            "wk_pack": wk_pack,
            "wvT": wvT,
            "bq_rep": bq_rep,
            "bk_pack": bk_pack,
            "gamma_bc": gm_bc,
        })

    nc = _get_program()
    res = run_bass_kernel_spmd(
        nc, in_maps, core_ids=list(range(8)),
        trace=bool(os.environ.get("BASS_TRACE")),
    )
    LAST_RESULT = res

    out = np.empty((B, C, N), dtype=np.float32)
    for core in range(8):
        b, h = core // 2, core % 2
        yT = res.results[core]["yT"]
        out[b][:, h * NH:(h + 1) * NH] = yT.T
    return out.reshape(B, C, H, W)
